# revision 7
# baseline (speedup 1.0000x reference)
"""Block-sparse attention Trainium2 kernel.

Reference model: nn.MultiheadAttention-style block-sparse attention,
B=1, L=4096, D=1024, H=16 heads, head_dim=64, block=128, global blocks
{0, 24} (block_idx % 24 == 0).

Sharding: head-parallel across 8 cores (2 heads/core). Each core:
  - projects full-sequence q, k, v for its 2 heads (transposed layouts)
  - computes the sparse attention for its 2 heads
  - computes its partial out-projection (contraction over its 128 head dims)
Host sums the 8 partial outputs (+ bias terms).

All heavy matmuls run in float32r (~1e-3 rel precision, 4x fp32 speed at
moving-dim >= 256).
"""

import sys

sys.path.insert(0, "/opt/trn_rl_repo")
import numpy as np

D = 1024
L = 4096
H = 16
HD = 64
NB = 32          # number of 128-row seq blocks
GLOB = (0, 24)   # global block indices
P = 128
SCALE = 1.0 / 8.0  # 1/sqrt(head_dim)

_CACHE = {}


def _build_nc(reps=1):
    import concourse.mybir as mybir
    import concourse.tile as tile
    from concourse import bacc
    from concourse.masks import make_identity

    f32 = mybir.dt.float32
    f32r = mybir.dt.float32r
    Act = mybir.ActivationFunctionType

    nc = bacc.Bacc("TRN2", target_bir_lowering=False, debug=False, num_devices=8)
    xT = nc.dram_tensor("xT", [D, L], f32, kind="ExternalInput")   # x.T
    wq = nc.dram_tensor("wq", [P, D], f32, kind="ExternalInput")   # (p, kt*128+o)
    wk = nc.dram_tensor("wk", [P, D], f32, kind="ExternalInput")
    wv = nc.dram_tensor("wv", [P, D], f32, kind="ExternalInput")
    wo = nc.dram_tensor("wo", [P, D], f32, kind="ExternalInput")   # w_out.T slice
    bq = nc.dram_tensor("bq", [P, 1], f32, kind="ExternalInput")
    bk = nc.dram_tensor("bk", [P, 1], f32, kind="ExternalInput")
    out = nc.dram_tensor("out", [L, D], f32, kind="ExternalOutput")

    with tile.TileContext(nc) as tc:
        with (
            tc.tile_pool(name="const", bufs=1) as constp,
            tc.tile_pool(name="stream", bufs=3) as streamp,
            tc.tile_pool(name="expb", bufs=6) as expp,
            tc.tile_pool(name="small", bufs=4) as smallp,
            tc.tile_pool(name="ps_big", bufs=3, space="PSUM") as ps_big,
            tc.tile_pool(name="ps_med", bufs=2, space="PSUM") as ps_med,
            tc.tile_pool(name="ps_av", bufs=2, space="PSUM") as ps_av,
            tc.tile_pool(name="ps_tr", bufs=1, space="PSUM") as ps_tr,
        ):
            # ---------- constants / persistent buffers
            ident = constp.tile([P, P], f32, tag="ident")
            make_identity(nc, ident[:])
            ones_col = constp.tile([P, 1], f32, tag="ones")
            nc.vector.memset(ones_col[:], 1.0)

            wq_r = constp.tile([P, D], f32r, tag="wq_r")
            wk_r = constp.tile([P, D], f32r, tag="wk_r")
            wv_r = constp.tile([P, D], f32r, tag="wv_r")
            wo_r = constp.tile([P, D], f32r, tag="wo_r")
            for dram, tr in ((wq, wq_r), (wk, wk_r), (wv, wv_r), (wo, wo_r)):
                raw = streamp.tile([P, D], f32, tag="wraw")
                nc.sync.dma_start(raw[:], dram[:])
                nc.vector.tensor_copy(tr[:], raw[:])
            bq_t = constp.tile([P, 1], f32, tag="bq")
            bk_t = constp.tile([P, 1], f32, tag="bk")
            nc.sync.dma_start(bq_t[:], bq[:])
            nc.sync.dma_start(bk_t[:], bk[:])

            qT = constp.tile([P, L], f32r, tag="qT")      # [2*64 hd, seq]
            kT = constp.tile([P, L], f32r, tag="kT")
            vTf = constp.tile([P, L], f32, tag="vTf")     # v transposed-proj (pre PE-transpose)
            vn = constp.tile([P, NB * 130], f32, tag="vn")  # per block: vA|1|vB|1
            qg = constp.tile([P, 256], f32r, tag="qg")    # staged q cols of global blocks

            import contextlib
            loop_ctx = tc.For_i(0, reps, 1) if reps > 1 else contextlib.nullcontext()
            with loop_ctx:
                _body(nc, tc, mybir, Act, f32, f32r, locals())

    nc.compile()
    return nc


def _body(nc, tc, mybir, Act, f32, f32r, env):
    constp = env["constp"]; streamp = env["streamp"]; expp = env["expp"]; smallp = env["smallp"]
    ps_big = env["ps_big"]; ps_med = env["ps_med"]; ps_av = env["ps_av"]; ps_tr = env["ps_tr"]
    ident = env["ident"]; ones_col = env["ones_col"]
    wq_r = env["wq_r"]; wk_r = env["wk_r"]; wv_r = env["wv_r"]; wo_r = env["wo_r"]
    bq_t = env["bq_t"]; bk_t = env["bk_t"]
    qT = env["qT"]; kT = env["kT"]; vTf = env["vTf"]; vn = env["vn"]; qg = env["qg"]
    xT = env["xT"]; out = env["out"]
    if True:
            # ---------- phase A: qkv projections (transposed outputs)
            # x streamed as [128,1024] tiles (quad = 1024 seq cols), cast to
            # f32r on gpsimd; 3 matmuls per (kt, 512-subchunk).
            for quad in range(4):
                xrs = []
                for kt in range(8):
                    xraw = streamp.tile([P, 1024], f32, tag="xraw")
                    nc.sync.dma_start(
                        xraw[:],
                        xT[kt * P:(kt + 1) * P, quad * 1024:(quad + 1) * 1024],
                    )
                    xr = streamp.tile([P, 1024], f32r, tag="xr", bufs=9)
                    nc.gpsimd.tensor_copy(xr[:], xraw[:])
                    xrs.append(xr)
                for sub in range(2):
                    n = quad * 2 + sub
                    psq = ps_big.tile([P, 512], f32, tag="psbig")
                    psk = ps_big.tile([P, 512], f32, tag="psbig")
                    psv = ps_big.tile([P, 512], f32, tag="psbig")
                    for kt in range(8):
                        st, sp = kt == 0, kt == 7
                        ks = slice(kt * P, (kt + 1) * P)
                        xsl = xrs[kt][:, sub * 512:(sub + 1) * 512]
                        nc.tensor.matmul(psq[:], wq_r[:, ks], xsl, start=st, stop=sp)
                        nc.tensor.matmul(psk[:], wk_r[:, ks], xsl, start=st, stop=sp)
                        nc.tensor.matmul(psv[:], wv_r[:, ks], xsl, start=st, stop=sp)
                    sl = slice(n * 512, (n + 1) * 512)
                    nc.scalar.activation(qT[:, sl], psq[:], Act.Identity, bias=bq_t[:])
                    nc.scalar.activation(kT[:, sl], psk[:], Act.Identity, bias=bk_t[:])
                    nc.scalar.activation(vTf[:, sl], psv[:], Act.Identity)

            # ---------- phase B: transpose v into natural layout (augmented with ones)
            for b in range(NB):
                pst = ps_tr.tile([P, P], f32, tag="pstr")
                nc.tensor.transpose(pst[:], vTf[:, b * P:(b + 1) * P], ident[:])
                base = b * 130
                nc.vector.tensor_copy(vn[:, base:base + 64], pst[:, 0:64])
                nc.vector.tensor_copy(vn[:, base + 65:base + 129], pst[:, 64:128])
                nc.vector.tensor_copy(vn[:, base + 64:base + 65], ones_col[:])
                nc.vector.tensor_copy(vn[:, base + 129:base + 130], ones_col[:])

            # stage global-q columns (qtiles 0 and 24 side by side)
            nc.vector.tensor_copy(qg[:, 0:128], qT[:, 0:128])
            nc.vector.tensor_copy(
                qg[:, 128:256], qT[:, GLOB[1] * P:(GLOB[1] + 1) * P]
            )

            def vslice(blk, h):
                return vn[:, blk * 130 + h * 65: blk * 130 + (h + 1) * 65]

            def finalize_qtile(j, acc_by_head):
                # acc_by_head[h]: [128 q, 65] (cols 0:64 = unnorm out, col 64 = l)
                onat = smallp.tile([P, P], f32, tag="onat")
                for h in (0, 1):
                    acc = acc_by_head[h]
                    linv = smallp.tile([P, 1], f32, tag="linv")
                    nc.vector.reciprocal(linv[:], acc[:, 64:65])
                    nc.vector.tensor_scalar_mul(
                        onat[:, h * 64:(h + 1) * 64], acc[:, 0:64], linv[:]
                    )
                ptt = ps_tr.tile([P, P], f32, tag="pstr")
                nc.tensor.transpose(ptt[:], onat[:], ident[:])
                otr = smallp.tile([P, P], f32r, tag="otr")
                nc.vector.tensor_copy(otr[:], ptt[:])
                osb = streamp.tile([P, D], f32, tag="osb")
                for half in (0, 1):
                    pso = ps_big.tile([P, 512], f32, tag="psbig")
                    nc.tensor.matmul(
                        pso[:], otr[:], wo_r[:, half * 512:(half + 1) * 512],
                        start=True, stop=True,
                    )
                    nc.vector.tensor_copy(
                        osb[:, half * 512:(half + 1) * 512], pso[:]
                    )
                nc.gpsimd.dma_start(out[j * P:(j + 1) * P, :], osb[:])

            # ---------- global qtiles (0 and 24): attend to all 32 blocks
            gacc = {}
            for qi in (0, 1):
                for h in (0, 1):
                    gacc[(qi, h)] = constp.tile([P, 65], f32, tag=f"gacc{qi}{h}", name=f"gacc{qi}{h}")
            AluAdd = mybir.AluOpType.add
            for kb2 in range(NB // 2):
                for h in (0, 1):
                    hs = slice(h * 64, (h + 1) * 64)
                    psg = ps_med.tile([P, 512], f32, tag="psmed")
                    for half in (0, 1):
                        kb = 2 * kb2 + half
                        nc.tensor.matmul(
                            psg[:, half * 256:(half + 1) * 256],
                            kT[hs, kb * P:(kb + 1) * P], qg[hs, :],
                            start=True, stop=True,
                        )
                    eg = expp.tile([P, 512], f32, tag="exp")
                    nc.scalar.activation(eg[:], psg[:], Act.Exp, scale=SCALE)
                    for half in (0, 1):
                        kb = 2 * kb2 + half
                        for qi in (0, 1):
                            pso = ps_av.tile([P, 65], f32, tag="psav")
                            nc.tensor.matmul(
                                pso[:],
                                eg[:, half * 256 + qi * 128: half * 256 + (qi + 1) * 128],
                                vslice(kb, h),
                                start=True, stop=True,
                            )
                            acc = gacc[(qi, h)]
                            if kb == 0:
                                nc.vector.tensor_copy(acc[:], pso[:])
                            else:
                                nc.vector.tensor_tensor(acc[:], acc[:], pso[:], AluAdd)
            finalize_qtile(0, {h: gacc[(0, h)] for h in (0, 1)})
            finalize_qtile(GLOB[1], {h: gacc[(1, h)] for h in (0, 1)})

            # ---------- regular qtiles, paired for wide score matmuls
            rest = [j for j in range(NB) if j not in GLOB]
            groups = []
            i = 0
            while i < len(rest):
                if i + 1 < len(rest) and rest[i + 1] == rest[i] + 1:
                    groups.append((rest[i], rest[i + 1]))
                    i += 2
                else:
                    groups.append((rest[i],))
                    i += 1

            for grp in groups:
                w = len(grp) * P
                q0 = grp[0] * P
                exps = {}
                for h in (0, 1):
                    hs = slice(h * 64, (h + 1) * 64)
                    rhs = qT[hs, q0:q0 + w]
                    for idx, j in enumerate(grp):
                        psd = ps_med.tile([P, 256], f32, tag="psmed")
                        nc.tensor.matmul(
                            psd[:, 0:w], kT[hs, j * P:(j + 1) * P], rhs,
                            start=True, stop=True,
                        )
                        ed = expp.tile([P, 256], f32, tag="exp")
                        nc.scalar.activation(
                            ed[:, 0:P], psd[:, idx * P:(idx + 1) * P],
                            Act.Exp, scale=SCALE,
                        )
                        exps[("d", j, h)] = ed
                    for g in GLOB:
                        psg = ps_med.tile([P, 256], f32, tag="psmed")
                        nc.tensor.matmul(
                            psg[:, 0:w], kT[hs, g * P:(g + 1) * P], rhs,
                            start=True, stop=True,
                        )
                        eg = expp.tile([P, 256], f32, tag="exp")
                        nc.scalar.activation(
                            eg[:, 0:w], psg[:, 0:w], Act.Exp, scale=SCALE
                        )
                        exps[("g", g, h)] = eg
                for idx, j in enumerate(grp):
                    accs = {}
                    for h in (0, 1):
                        pso = ps_av.tile([P, 65], f32, tag="psav")
                        nc.tensor.matmul(
                            pso[:], exps[("d", j, h)][:, 0:P], vslice(j, h),
                            start=True, stop=False,
                        )
                        nc.tensor.matmul(
                            pso[:],
                            exps[("g", GLOB[0], h)][:, idx * P:(idx + 1) * P],
                            vslice(GLOB[0], h), start=False, stop=False,
                        )
                        nc.tensor.matmul(
                            pso[:],
                            exps[("g", GLOB[1], h)][:, idx * P:(idx + 1) * P],
                            vslice(GLOB[1], h), start=False, stop=True,
                        )
                        accs[h] = pso
                    finalize_qtile(j, accs)


def _get_nc(reps=1):
    key = ("nc", reps)
    if key not in _CACHE:
        _CACHE[key] = _build_nc(reps)
    return _CACHE[key]


def _prep_inputs(x, w_qkv, b_qkv):
    x2 = np.asarray(x, dtype=np.float32).reshape(L, D)
    xT = np.ascontiguousarray(x2.T)
    w_qkv = np.asarray(w_qkv, dtype=np.float32)
    b_qkv = np.asarray(b_qkv, dtype=np.float32)

    def tile_w(w_slice):
        # [128 out, 1024 in] -> [128 p, 8 kt, 128 o] flattened
        wt = w_slice.T  # [1024 in, 128 out]
        return np.ascontiguousarray(
            wt.reshape(8, P, P).transpose(1, 0, 2).reshape(P, D)
        )

    maps = []
    for c in range(8):
        a = 2 * c * HD
        b = a + 2 * HD
        maps.append({
            "xT": xT,
            "wq": tile_w(w_qkv[a:b, :]),
            "wk": tile_w(w_qkv[D + a:D + b, :]),
            "wv": tile_w(w_qkv[2 * D + a:2 * D + b, :]),
            "bq": np.ascontiguousarray(b_qkv[a:b].reshape(P, 1)),
            "bk": np.ascontiguousarray(b_qkv[D + a:D + b].reshape(P, 1)),
        })
    return maps


def kernel(x, w_qkv, b_qkv, w_out, b_out):
    from concourse.bass_utils import run_bass_kernel_spmd

    x = np.asarray(x, dtype=np.float32)
    w_qkv = np.asarray(w_qkv, dtype=np.float32)
    b_qkv = np.asarray(b_qkv, dtype=np.float32)
    w_out = np.asarray(w_out, dtype=np.float32)
    b_out = np.asarray(b_out, dtype=np.float32)

    nc = _get_nc()
    maps = _prep_inputs(x, w_qkv, b_qkv)
    for c in range(8):
        a = 2 * c * HD
        b = a + 2 * HD
        maps[c]["wo"] = np.ascontiguousarray(w_out[:, a:b].T)

    res = run_bass_kernel_spmd(nc, maps, core_ids=list(range(8)))

    total = res.results[0]["out"].copy()
    for c in range(1, 8):
        total += res.results[c]["out"]
    # v-bias contributes (sum_k attn = 1) * b_v @ w_out.T; plus out bias.
    const_row = b_qkv[2 * D:3 * D] @ w_out.T + b_out
    total += const_row[None, :]
    return total.reshape(x.shape).astype(np.float32)


# revision 8
# speedup vs baseline: 1.0389x; 1.0389x over previous
"""Block-sparse attention Trainium2 kernel.

Reference model: nn.MultiheadAttention-style block-sparse attention,
B=1, L=4096, D=1024, H=16 heads, head_dim=64, block=128, global blocks
{0, 24} (block_idx % 24 == 0).

Sharding: head-parallel across 8 cores (2 heads/core). Each core:
  - projects full-sequence q, k, v for its 2 heads (transposed layouts)
  - computes the sparse attention for its 2 heads
  - computes its partial out-projection (contraction over its 128 head dims)
Host sums the 8 partial outputs (+ bias terms).

All heavy matmuls run in float32r (~1e-3 rel precision, 4x fp32 speed at
moving-dim >= 256).
"""

import sys

sys.path.insert(0, "/opt/trn_rl_repo")
import numpy as np

D = 1024
L = 4096
H = 16
HD = 64
NB = 32          # number of 128-row seq blocks
GLOB = (0, 24)   # global block indices
P = 128
SCALE = 1.0 / 8.0  # 1/sqrt(head_dim)

_CACHE = {}


def _build_nc(reps=1):
    import concourse.mybir as mybir
    import concourse.tile as tile
    from concourse import bacc
    from concourse.masks import make_identity

    f32 = mybir.dt.float32
    f32r = mybir.dt.float32r
    Act = mybir.ActivationFunctionType

    nc = bacc.Bacc("TRN2", target_bir_lowering=False, debug=False, num_devices=8)
    xT = nc.dram_tensor("xT", [D, L], f32, kind="ExternalInput")   # x.T
    wq = nc.dram_tensor("wq", [P, D], f32, kind="ExternalInput")   # (p, kt*128+o)
    wk = nc.dram_tensor("wk", [P, D], f32, kind="ExternalInput")
    wv = nc.dram_tensor("wv", [P, D], f32, kind="ExternalInput")
    wo = nc.dram_tensor("wo", [P, D], f32, kind="ExternalInput")   # w_out.T slice
    bq = nc.dram_tensor("bq", [P, 1], f32, kind="ExternalInput")
    bk = nc.dram_tensor("bk", [P, 1], f32, kind="ExternalInput")
    out = nc.dram_tensor("out", [L, D], f32, kind="ExternalOutput")

    with tile.TileContext(nc) as tc:
        with (
            tc.tile_pool(name="const", bufs=1) as constp,
            tc.tile_pool(name="stream", bufs=3) as streamp,
            tc.tile_pool(name="expb", bufs=6) as expp,
            tc.tile_pool(name="small", bufs=4) as smallp,
            tc.tile_pool(name="ps_big", bufs=3, space="PSUM") as ps_big,
            tc.tile_pool(name="ps_med", bufs=2, space="PSUM") as ps_med,
            tc.tile_pool(name="ps_av", bufs=2, space="PSUM") as ps_av,
            tc.tile_pool(name="ps_tr", bufs=1, space="PSUM") as ps_tr,
        ):
            # ---------- constants / persistent buffers
            ident = constp.tile([P, P], f32, tag="ident")
            make_identity(nc, ident[:])
            ones_col = constp.tile([P, 1], f32, tag="ones")
            nc.vector.memset(ones_col[:], 1.0)

            wq_r = constp.tile([P, D], f32r, tag="wq_r")
            wk_r = constp.tile([P, D], f32r, tag="wk_r")
            wv_r = constp.tile([P, D], f32r, tag="wv_r")
            wo_r = constp.tile([P, D], f32r, tag="wo_r")
            for dram, tr in ((wq, wq_r), (wk, wk_r), (wv, wv_r), (wo, wo_r)):
                raw = streamp.tile([P, D], f32, tag="wraw")
                nc.sync.dma_start(raw[:], dram[:])
                nc.vector.tensor_copy(tr[:], raw[:])
            bq_t = constp.tile([P, 1], f32, tag="bq")
            bk_t = constp.tile([P, 1], f32, tag="bk")
            nc.sync.dma_start(bq_t[:], bq[:])
            nc.sync.dma_start(bk_t[:], bk[:])

            qT = constp.tile([P, L], f32r, tag="qT")      # [2*64 hd, seq]
            kT = constp.tile([P, L], f32r, tag="kT")
            vTf = constp.tile([P, L], f32, tag="vTf")     # v transposed-proj (pre PE-transpose)
            vn = constp.tile([P, NB * 130], f32, tag="vn")  # per block: vA|1|vB|1
            qg = constp.tile([P, 256], f32r, tag="qg")    # staged q cols of global blocks

            import contextlib
            loop_ctx = tc.For_i(0, reps, 1) if reps > 1 else contextlib.nullcontext()
            with loop_ctx:
                _body(nc, tc, mybir, Act, f32, f32r, locals())

    nc.compile()
    return nc


def _body(nc, tc, mybir, Act, f32, f32r, env):
    constp = env["constp"]; streamp = env["streamp"]; expp = env["expp"]; smallp = env["smallp"]
    ps_big = env["ps_big"]; ps_med = env["ps_med"]; ps_av = env["ps_av"]; ps_tr = env["ps_tr"]
    ident = env["ident"]; ones_col = env["ones_col"]
    wq_r = env["wq_r"]; wk_r = env["wk_r"]; wv_r = env["wv_r"]; wo_r = env["wo_r"]
    bq_t = env["bq_t"]; bk_t = env["bk_t"]
    qT = env["qT"]; kT = env["kT"]; vTf = env["vTf"]; vn = env["vn"]; qg = env["qg"]
    xT = env["xT"]; out = env["out"]
    if True:
            # ---------- phase A: qkv projections (transposed outputs)
            # x streamed as [128,1024] tiles (quad = 1024 seq cols), cast to
            # f32r on gpsimd; 3 matmuls per (kt, 512-subchunk).
            for quad in range(4):
                xrs = []
                for kt in range(8):
                    xraw = streamp.tile([P, 1024], f32, tag="xraw")
                    nc.sync.dma_start(
                        xraw[:],
                        xT[kt * P:(kt + 1) * P, quad * 1024:(quad + 1) * 1024],
                    )
                    xr = streamp.tile([P, 1024], f32r, tag="xr", bufs=9)
                    nc.gpsimd.tensor_copy(xr[:], xraw[:])
                    xrs.append(xr)
                for sub in range(2):
                    n = quad * 2 + sub
                    psq = ps_big.tile([P, 512], f32, tag="psbig")
                    psk = ps_big.tile([P, 512], f32, tag="psbig")
                    psv = ps_big.tile([P, 512], f32, tag="psbig")
                    for kt in range(8):
                        st, sp = kt == 0, kt == 7
                        ks = slice(kt * P, (kt + 1) * P)
                        xsl = xrs[kt][:, sub * 512:(sub + 1) * 512]
                        nc.tensor.matmul(psq[:], wq_r[:, ks], xsl, start=st, stop=sp)
                        nc.tensor.matmul(psk[:], wk_r[:, ks], xsl, start=st, stop=sp)
                        nc.tensor.matmul(psv[:], wv_r[:, ks], xsl, start=st, stop=sp)
                    sl = slice(n * 512, (n + 1) * 512)
                    nc.scalar.activation(qT[:, sl], psq[:], Act.Identity, bias=bq_t[:])
                    nc.scalar.activation(kT[:, sl], psk[:], Act.Identity, bias=bk_t[:])
                    nc.scalar.activation(vTf[:, sl], psv[:], Act.Identity)

            # ---------- phase B: transpose v into natural layout (augmented with ones)
            for b in range(NB):
                pst = ps_tr.tile([P, P], f32, tag="pstr")
                nc.tensor.transpose(pst[:], vTf[:, b * P:(b + 1) * P], ident[:])
                base = b * 130
                nc.vector.tensor_copy(vn[:, base:base + 64], pst[:, 0:64])
                nc.vector.tensor_copy(vn[:, base + 65:base + 129], pst[:, 64:128])
                nc.vector.tensor_copy(vn[:, base + 64:base + 65], ones_col[:])
                nc.vector.tensor_copy(vn[:, base + 129:base + 130], ones_col[:])

            # stage global-q columns (qtiles 0 and 24 side by side)
            nc.vector.tensor_copy(qg[:, 0:128], qT[:, 0:128])
            nc.vector.tensor_copy(
                qg[:, 128:256], qT[:, GLOB[1] * P:(GLOB[1] + 1) * P]
            )

            def vslice(blk, h):
                return vn[:, blk * 130 + h * 65: blk * 130 + (h + 1) * 65]

            def finalize_qtile(j, acc_by_head):
                # acc_by_head[h]: [128 q, 65] (cols 0:64 = unnorm out, col 64 = l)
                onat = smallp.tile([P, P], f32, tag="onat")
                for h in (0, 1):
                    acc = acc_by_head[h]
                    linv = smallp.tile([P, 1], f32, tag="linv")
                    nc.vector.reciprocal(linv[:], acc[:, 64:65])
                    nc.vector.tensor_scalar_mul(
                        onat[:, h * 64:(h + 1) * 64], acc[:, 0:64], linv[:]
                    )
                ptt = ps_tr.tile([P, P], f32, tag="pstr")
                nc.tensor.transpose(ptt[:], onat[:], ident[:])
                otr = smallp.tile([P, P], f32r, tag="otr")
                nc.vector.tensor_copy(otr[:], ptt[:])
                osb = streamp.tile([P, D], f32, tag="osb")
                for half in (0, 1):
                    pso = ps_big.tile([P, 512], f32, tag="psbig")
                    nc.tensor.matmul(
                        pso[:], otr[:], wo_r[:, half * 512:(half + 1) * 512],
                        start=True, stop=True,
                    )
                    nc.vector.tensor_copy(
                        osb[:, half * 512:(half + 1) * 512], pso[:]
                    )
                nc.sync.dma_start(out[j * P:(j + 1) * P, :], osb[:])

            # ---------- global qtiles (0 and 24): attend to all 32 blocks
            gacc = {}
            for qi in (0, 1):
                for h in (0, 1):
                    gacc[(qi, h)] = constp.tile([P, 65], f32, tag=f"gacc{qi}{h}", name=f"gacc{qi}{h}")
            AluAdd = mybir.AluOpType.add
            for kb2 in range(NB // 2):
                for h in (0, 1):
                    hs = slice(h * 64, (h + 1) * 64)
                    psg = ps_med.tile([P, 512], f32, tag="psmed")
                    for half in (0, 1):
                        kb = 2 * kb2 + half
                        nc.tensor.matmul(
                            psg[:, half * 256:(half + 1) * 256],
                            kT[hs, kb * P:(kb + 1) * P], qg[hs, :],
                            start=True, stop=True,
                        )
                    eg = expp.tile([P, 512], f32, tag="exp")
                    nc.scalar.activation(eg[:], psg[:], Act.Exp, scale=SCALE)
                    for half in (0, 1):
                        kb = 2 * kb2 + half
                        for qi in (0, 1):
                            pso = ps_av.tile([P, 65], f32, tag="psav")
                            nc.tensor.matmul(
                                pso[:],
                                eg[:, half * 256 + qi * 128: half * 256 + (qi + 1) * 128],
                                vslice(kb, h),
                                start=True, stop=True,
                            )
                            acc = gacc[(qi, h)]
                            if kb == 0:
                                nc.vector.tensor_copy(acc[:], pso[:])
                            else:
                                nc.vector.tensor_tensor(acc[:], acc[:], pso[:], AluAdd)
            finalize_qtile(0, {h: gacc[(0, h)] for h in (0, 1)})
            finalize_qtile(GLOB[1], {h: gacc[(1, h)] for h in (0, 1)})

            # ---------- regular qtiles, paired for wide score matmuls
            rest = [j for j in range(NB) if j not in GLOB]
            groups = []
            i = 0
            while i < len(rest):
                if i + 1 < len(rest) and rest[i + 1] == rest[i] + 1:
                    groups.append((rest[i], rest[i + 1]))
                    i += 2
                else:
                    groups.append((rest[i],))
                    i += 1

            for grp in groups:
                w = len(grp) * P
                q0 = grp[0] * P
                exps = {}
                for h in (0, 1):
                    hs = slice(h * 64, (h + 1) * 64)
                    rhs = qT[hs, q0:q0 + w]
                    for idx, j in enumerate(grp):
                        psd = ps_med.tile([P, 256], f32, tag="psmed")
                        nc.tensor.matmul(
                            psd[:, 0:w], kT[hs, j * P:(j + 1) * P], rhs,
                            start=True, stop=True,
                        )
                        ed = expp.tile([P, 256], f32, tag="exp")
                        nc.scalar.activation(
                            ed[:, 0:P], psd[:, idx * P:(idx + 1) * P],
                            Act.Exp, scale=SCALE,
                        )
                        exps[("d", j, h)] = ed
                    for g in GLOB:
                        psg = ps_med.tile([P, 256], f32, tag="psmed")
                        nc.tensor.matmul(
                            psg[:, 0:w], kT[hs, g * P:(g + 1) * P], rhs,
                            start=True, stop=True,
                        )
                        eg = expp.tile([P, 256], f32, tag="exp")
                        nc.scalar.activation(
                            eg[:, 0:w], psg[:, 0:w], Act.Exp, scale=SCALE
                        )
                        exps[("g", g, h)] = eg
                for idx, j in enumerate(grp):
                    accs = {}
                    for h in (0, 1):
                        pso = ps_av.tile([P, 65], f32, tag="psav")
                        nc.tensor.matmul(
                            pso[:], exps[("d", j, h)][:, 0:P], vslice(j, h),
                            start=True, stop=False,
                        )
                        nc.tensor.matmul(
                            pso[:],
                            exps[("g", GLOB[0], h)][:, idx * P:(idx + 1) * P],
                            vslice(GLOB[0], h), start=False, stop=False,
                        )
                        nc.tensor.matmul(
                            pso[:],
                            exps[("g", GLOB[1], h)][:, idx * P:(idx + 1) * P],
                            vslice(GLOB[1], h), start=False, stop=True,
                        )
                        accs[h] = pso
                    finalize_qtile(j, accs)


def _get_nc(reps=1):
    key = ("nc", reps)
    if key not in _CACHE:
        _CACHE[key] = _build_nc(reps)
    return _CACHE[key]


def _prep_inputs(x, w_qkv, b_qkv):
    x2 = np.asarray(x, dtype=np.float32).reshape(L, D)
    xT = np.ascontiguousarray(x2.T)
    w_qkv = np.asarray(w_qkv, dtype=np.float32)
    b_qkv = np.asarray(b_qkv, dtype=np.float32)

    def tile_w(w_slice):
        # [128 out, 1024 in] -> [128 p, 8 kt, 128 o] flattened
        wt = w_slice.T  # [1024 in, 128 out]
        return np.ascontiguousarray(
            wt.reshape(8, P, P).transpose(1, 0, 2).reshape(P, D)
        )

    maps = []
    for c in range(8):
        a = 2 * c * HD
        b = a + 2 * HD
        maps.append({
            "xT": xT,
            "wq": tile_w(w_qkv[a:b, :]),
            "wk": tile_w(w_qkv[D + a:D + b, :]),
            "wv": tile_w(w_qkv[2 * D + a:2 * D + b, :]),
            "bq": np.ascontiguousarray(b_qkv[a:b].reshape(P, 1)),
            "bk": np.ascontiguousarray(b_qkv[D + a:D + b].reshape(P, 1)),
        })
    return maps


def kernel(x, w_qkv, b_qkv, w_out, b_out):
    from concourse.bass_utils import run_bass_kernel_spmd

    x = np.asarray(x, dtype=np.float32)
    w_qkv = np.asarray(w_qkv, dtype=np.float32)
    b_qkv = np.asarray(b_qkv, dtype=np.float32)
    w_out = np.asarray(w_out, dtype=np.float32)
    b_out = np.asarray(b_out, dtype=np.float32)

    nc = _get_nc()
    maps = _prep_inputs(x, w_qkv, b_qkv)
    for c in range(8):
        a = 2 * c * HD
        b = a + 2 * HD
        maps[c]["wo"] = np.ascontiguousarray(w_out[:, a:b].T)

    res = run_bass_kernel_spmd(nc, maps, core_ids=list(range(8)))

    total = res.results[0]["out"].copy()
    for c in range(1, 8):
        total += res.results[c]["out"]
    # v-bias contributes (sum_k attn = 1) * b_v @ w_out.T; plus out bias.
    const_row = b_qkv[2 * D:3 * D] @ w_out.T + b_out
    total += const_row[None, :]
    return total.reshape(x.shape).astype(np.float32)


# revision 13
# speedup vs baseline: 1.3847x; 1.3329x over previous
"""Block-sparse attention Trainium2 kernel (v2, transposed-AV).

Reference: nn.MultiheadAttention-style block-sparse attention, B=1, L=4096,
D=1024, H=16, head_dim=64, block=128, global blocks {0, 24}.

Sharding: head-parallel across 8 cores (2 heads/core); host sums the 8
partial out-projections. All wide matmuls in float32r; attention-value
products computed in transposed form (outT = v_aug.T @ expT) so every AV
matmul has a wide moving dim; softmax denominators ride along as row 64 of
the augmented V; normalization is a reciprocal + PE outer-product broadcast +
one elementwise multiply per 512-wide chunk.
"""

import sys

sys.path.insert(0, "/opt/trn_rl_repo")
import numpy as np

D = 1024
L = 4096
H = 16
HD = 64
NB = 32
GLOB = (0, 24)
P = 128
SCALE = 1.0 / 8.0

_CACHE = {}


def _build_nc(reps=1):
    import contextlib

    import concourse.mybir as mybir
    import concourse.tile as tile
    from concourse import bacc
    from concourse.masks import make_identity

    f32 = mybir.dt.float32
    f32r = mybir.dt.float32r
    Act = mybir.ActivationFunctionType
    AluMult = mybir.AluOpType.mult

    nc = bacc.Bacc("TRN2", target_bir_lowering=False, debug=False, num_devices=8)
    xT = nc.dram_tensor("xT", [D, L], f32, kind="ExternalInput")
    wq = nc.dram_tensor("wq", [P, D], f32, kind="ExternalInput")
    wk = nc.dram_tensor("wk", [P, D], f32, kind="ExternalInput")
    wv = nc.dram_tensor("wv", [P, D], f32, kind="ExternalInput")
    wo = nc.dram_tensor("wo", [P, D], f32, kind="ExternalInput")
    bq = nc.dram_tensor("bq", [P, 1], f32, kind="ExternalInput")
    bk = nc.dram_tensor("bk", [P, 1], f32, kind="ExternalInput")
    out = nc.dram_tensor("out", [L, D], f32, kind="ExternalOutput")

    with tile.TileContext(nc) as tc:
        with (
            tc.tile_pool(name="const", bufs=1) as constp,
            tc.tile_pool(name="stream", bufs=3) as streamp,
            tc.tile_pool(name="expb", bufs=6) as expp,
            tc.tile_pool(name="small", bufs=4) as smallp,
            tc.tile_pool(name="ps_big", bufs=2, space="PSUM") as ps_big,
            tc.tile_pool(name="ps_med", bufs=2, space="PSUM") as ps_med,
            tc.tile_pool(name="ps_av", bufs=3, space="PSUM") as ps_av,
        ):
            # ---------- constants / persistent buffers
            ident = constp.tile([P, P], f32, tag="ident")
            make_identity(nc, ident[:])
            ones_col = constp.tile([P, 1], f32, tag="ones")
            nc.vector.memset(ones_col[:], 1.0)
            ones_row_f = constp.tile([1, 64], f32, tag="onesrf")
            nc.vector.memset(ones_row_f[:], 1.0)
            ones_row = constp.tile([1, 64], f32r, tag="onesr")
            nc.vector.tensor_copy(ones_row[:], ones_row_f[:])

            wq_r = constp.tile([P, D], f32r, tag="wq_r")
            wk_r = constp.tile([P, D], f32r, tag="wk_r")
            wv_r = constp.tile([P, D], f32r, tag="wv_r")
            wo_r = constp.tile([P, D], f32r, tag="wo_r")
            for dram, tr in ((wq, wq_r), (wk, wk_r), (wv, wv_r), (wo, wo_r)):
                raw = streamp.tile([P, D], f32, tag="wraw")
                nc.sync.dma_start(raw[:], dram[:])
                nc.vector.tensor_copy(tr[:], raw[:])
            bq_t = constp.tile([P, 1], f32, tag="bq")
            bk_t = constp.tile([P, 1], f32, tag="bk")
            nc.sync.dma_start(bq_t[:], bq[:])
            nc.sync.dma_start(bk_t[:], bk[:])

            qT = constp.tile([P, L], f32r, tag="qT")
            kT = constp.tile([P, L], f32r, tag="kT")
            vTf = constp.tile([P, L], f32, tag="vTf")
            vn = constp.tile([P, NB * 130], f32r, tag="vn")
            qg = constp.tile([P, 256], f32r, tag="qg")
            gout = constp.tile([P, 256], f32r, tag="gout")

            loop_ctx = tc.For_i(0, reps, 1) if reps > 1 else contextlib.nullcontext()
            with loop_ctx:
                _body(nc, tc, mybir, Act, f32, f32r, AluMult, locals())

    nc.compile()
    return nc


def _body(nc, tc, mybir, Act, f32, f32r, AluMult, env):
    constp = env["constp"]; streamp = env["streamp"]; expp = env["expp"]; smallp = env["smallp"]
    ps_big = env["ps_big"]; ps_med = env["ps_med"]; ps_av = env["ps_av"]
    ident = env["ident"]; ones_col = env["ones_col"]; ones_row = env["ones_row"]
    wq_r = env["wq_r"]; wk_r = env["wk_r"]; wv_r = env["wv_r"]; wo_r = env["wo_r"]
    bq_t = env["bq_t"]; bk_t = env["bk_t"]
    qT = env["qT"]; kT = env["kT"]; vTf = env["vTf"]; vn = env["vn"]
    qg = env["qg"]; gout = env["gout"]
    xT = env["xT"]; out = env["out"]

    # ---------- phase A: qkv projections (transposed outputs)
    for quad in range(4):
        xrs = []
        for kt in range(8):
            xraw = streamp.tile([P, 1024], f32, tag="xraw")
            nc.sync.dma_start(
                xraw[:], xT[kt * P:(kt + 1) * P, quad * 1024:(quad + 1) * 1024]
            )
            xr = streamp.tile([P, 1024], f32r, tag="xr", bufs=9)
            nc.gpsimd.tensor_copy(xr[:], xraw[:])
            xrs.append(xr)
        for sub in range(2):
            n = quad * 2 + sub
            sl = slice(n * 512, (n + 1) * 512)
            for wt, dest, bias in (
                (wq_r, qT, bq_t),
                (wk_r, kT, bk_t),
                (wv_r, vTf, None),
            ):
                pp = ps_big.tile([P, 512], f32, tag="psbig")
                for kt in range(8):
                    nc.tensor.matmul(
                        pp[:], wt[:, kt * P:(kt + 1) * P],
                        xrs[kt][:, sub * 512:(sub + 1) * 512],
                        start=kt == 0, stop=kt == 7,
                    )
                if bias is not None:
                    nc.scalar.activation(dest[:, sl], pp[:], Act.Identity, bias=bias[:])
                else:
                    nc.scalar.activation(dest[:, sl], pp[:], Act.Identity)

    # ---------- phase B: transpose v into augmented natural layout
    for b in range(NB):
        pst = ps_av.tile([P, P], f32, tag="psav", name=f"pst{b}")
        nc.tensor.transpose(pst[:], vTf[:, b * P:(b + 1) * P], ident[:])
        base = b * 130
        nc.vector.tensor_copy(vn[:, base:base + 64], pst[:, 0:64])
        nc.vector.tensor_copy(vn[:, base + 65:base + 129], pst[:, 64:128])
        nc.vector.tensor_copy(vn[:, base + 64:base + 65], ones_col[:])
        nc.vector.tensor_copy(vn[:, base + 129:base + 130], ones_col[:])

    # stage global-q columns (qtiles 0 and 24 side by side)
    nc.vector.tensor_copy(qg[:, 0:128], qT[:, 0:128])
    nc.vector.tensor_copy(qg[:, 128:256], qT[:, GLOB[1] * P:(GLOB[1] + 1) * P])

    def vslice(blk, h):
        return vn[:, blk * 130 + h * 65: blk * 130 + (h + 1) * 65]

    def normalize_emit(psumT, lo, hi, dest):
        # psumT [65, W+]: rows 0:64 = unnormalized outT, row 64 = l
        W = hi - lo
        linv = smallp.tile([1, 512], f32r, tag="linv")
        with nc.allow_low_precision(reason="f32r has near-f32 mantissa here"):
            nc.vector.reciprocal(linv[0:1, 0:W], psumT[64:65, lo:hi])
        psb = ps_av.tile([64, 512], f32, tag="psav")
        nc.tensor.matmul(psb[0:64, 0:W], ones_row[:], linv[0:1, 0:W],
                         start=True, stop=True)
        bsb = smallp.tile([64, 512], f32, tag="bsb")
        nc.vector.tensor_copy(bsb[0:64, 0:W], psb[0:64, 0:W])
        nc.vector.tensor_tensor(dest, psumT[0:64, lo:hi], bsb[0:64, 0:W], AluMult)

    # ---------- global qtiles (0 and 24): attend to all 32 blocks
    for h in (0, 1):
        hs = slice(h * 64, (h + 1) * 64)
        pg = ps_med.tile([65, 256], f32, tag="gt", bufs=1, name=f"pg{h}")
        for kb2 in range(NB // 2):
            psg = ps_med.tile([P, 512], f32, tag="psmed")
            for half in (0, 1):
                kb = 2 * kb2 + half
                nc.tensor.matmul(
                    psg[:, half * 256:(half + 1) * 256],
                    kT[hs, kb * P:(kb + 1) * P], qg[hs, :],
                    start=True, stop=True,
                )
            eg = expp.tile([P, 512], f32r, tag="exp")
            nc.scalar.activation(eg[:], psg[:], Act.Exp, scale=SCALE)
            for half in (0, 1):
                kb = 2 * kb2 + half
                nc.tensor.matmul(
                    pg[:], vslice(kb, h), eg[:, half * 256:(half + 1) * 256],
                    start=kb == 0, stop=kb == NB - 1,
                )
        normalize_emit(pg, 0, 256, gout[h * 64:(h + 1) * 64, :])

    # ---------- chunk loop: 8 chunks of 512 q-cols (4 qtiles each)
    for c in range(8):
        otr = smallp.tile([P, 512], f32r, tag="otr", name=f"otr{c}")
        glob_in_chunk = [g for g in GLOB if g // 4 == c]
        lo = 128 if glob_in_chunk else 0
        qts = [4 * c + i for i in range(4) if (4 * c + i) not in GLOB]
        if glob_in_chunk:
            g = glob_in_chunk[0]
            gq_col = 0 if g == 0 else 128
            nc.vector.tensor_copy(otr[:, 0:128], gout[:, gq_col:gq_col + 128])
        for h in (0, 1):
            hs = slice(h * 64, (h + 1) * 64)
            # global-kblock scores over the whole chunk
            egs = {}
            for g in GLOB:
                psg = ps_med.tile([P, 512], f32, tag="psmed")
                nc.tensor.matmul(
                    psg[:], kT[hs, g * P:(g + 1) * P],
                    qT[hs, c * 512:(c + 1) * 512],
                    start=True, stop=True,
                )
                eg = expp.tile([P, 512], f32r, tag="exp")
                nc.scalar.activation(eg[:], psg[:], Act.Exp, scale=SCALE)
                egs[g] = eg
            # diagonal scores, pairs of qtiles where possible
            groups = []
            i = 0
            while i < len(qts):
                if i + 1 < len(qts) and qts[i + 1] == qts[i] + 1:
                    groups.append((qts[i], qts[i + 1])); i += 2
                else:
                    groups.append((qts[i],)); i += 1
            eds = {}
            for grp in groups:
                w = len(grp) * P
                q0 = grp[0] * P
                for idx, j in enumerate(grp):
                    psd = ps_med.tile([P, 256], f32, tag="psmed")
                    nc.tensor.matmul(
                        psd[:, 0:w], kT[hs, j * P:(j + 1) * P], qT[hs, q0:q0 + w],
                        start=True, stop=True,
                    )
                    ed = expp.tile([P, 256], f32r, tag="exp")
                    nc.scalar.activation(
                        ed[:, 0:P], psd[:, idx * P:(idx + 1) * P],
                        Act.Exp, scale=SCALE,
                    )
                    eds[j] = ed
            # transposed AV accumulation into [65, 512] chunk psum
            pc = ps_av.tile([65, 512], f32, tag="psav")
            nc.tensor.matmul(pc[:, lo:512], vslice(GLOB[0], h),
                             egs[GLOB[0]][:, lo:512], start=True, stop=False)
            nc.tensor.matmul(pc[:, lo:512], vslice(GLOB[1], h),
                             egs[GLOB[1]][:, lo:512], start=False, stop=True)
            for j in qts:
                off = (j - 4 * c) * P
                nc.tensor.matmul(pc[:, off:off + P], vslice(j, h),
                                 eds[j][:, 0:P], start=False, stop=True,
                                 skip_group_check=True)  # sub-region accumulate
            normalize_emit(pc, lo, 512, otr[h * 64:(h + 1) * 64, lo:512])
        # out-projection + store, per seq-tile
        for t in range(4):
            j = 4 * c + t
            osb = streamp.tile([P, D], f32, tag="osb")
            for half in (0, 1):
                pso = ps_big.tile([P, 512], f32, tag="psbig")
                nc.tensor.matmul(
                    pso[:], otr[:, t * P:(t + 1) * P],
                    wo_r[:, half * 512:(half + 1) * 512],
                    start=True, stop=True,
                )
                nc.vector.tensor_copy(osb[:, half * 512:(half + 1) * 512], pso[:])
            nc.sync.dma_start(out[j * P:(j + 1) * P, :], osb[:])


def _get_nc(reps=1):
    key = ("nc", reps)
    if key not in _CACHE:
        _CACHE[key] = _build_nc(reps)
    return _CACHE[key]


def _prep_inputs(x, w_qkv, b_qkv):
    x2 = np.asarray(x, dtype=np.float32).reshape(L, D)
    xT = np.ascontiguousarray(x2.T)
    w_qkv = np.asarray(w_qkv, dtype=np.float32)
    b_qkv = np.asarray(b_qkv, dtype=np.float32)

    def tile_w(w_slice):
        wt = w_slice.T
        return np.ascontiguousarray(
            wt.reshape(8, P, P).transpose(1, 0, 2).reshape(P, D)
        )

    maps = []
    for c in range(8):
        a = 2 * c * HD
        b = a + 2 * HD
        maps.append({
            "xT": xT,
            "wq": tile_w(w_qkv[a:b, :]),
            "wk": tile_w(w_qkv[D + a:D + b, :]),
            "wv": tile_w(w_qkv[2 * D + a:2 * D + b, :]),
            "bq": np.ascontiguousarray(b_qkv[a:b].reshape(P, 1)),
            "bk": np.ascontiguousarray(b_qkv[D + a:D + b].reshape(P, 1)),
        })
    return maps


def kernel(x, w_qkv, b_qkv, w_out, b_out):
    from concourse.bass_utils import run_bass_kernel_spmd

    x = np.asarray(x, dtype=np.float32)
    w_qkv = np.asarray(w_qkv, dtype=np.float32)
    b_qkv = np.asarray(b_qkv, dtype=np.float32)
    w_out = np.asarray(w_out, dtype=np.float32)
    b_out = np.asarray(b_out, dtype=np.float32)

    nc = _get_nc()
    maps = _prep_inputs(x, w_qkv, b_qkv)
    for c in range(8):
        a = 2 * c * HD
        b = a + 2 * HD
        maps[c]["wo"] = np.ascontiguousarray(w_out[:, a:b].T)

    res = run_bass_kernel_spmd(nc, maps, core_ids=list(range(8)))

    total = res.results[0]["out"].copy()
    for c in range(1, 8):
        total += res.results[c]["out"]
    const_row = b_qkv[2 * D:3 * D] @ w_out.T + b_out
    total += const_row[None, :]
    return total.reshape(x.shape).astype(np.float32)
